# revision 1
# baseline (speedup 1.0000x reference)
"""Trainium2 Bass kernel for FFTConv: y = tanh(ifft(fft(u)*fft(k)).real + diag(D)*u).

Shapes: u (8,256,16384) f32, k (256,16384) f32, D (256,256) f32.
Strategy: shard H across 8 cores (32 channels each). Per (b,h) sequence of
length L=16384=128*128, compute the circular convolution via a four-step FFT:
both DFT stages are 128x128 matmuls on the tensor engine, twiddle/spectrum
pointwise stages run in fp16 on DVE/GPSIMD, PSUM->SBUF converts + final tanh
on the scalar engine. The diag(D) feedthrough is folded into the last matmul
as a diagonal-stationary accumulate.
"""

import numpy as np

B, H, L = 8, 256, 16384
N = 128
HSH = H // 8  # 32 channels per core

_CACHE = {}


def _consts():
    n = np.arange(N)
    F1 = np.exp(-2j * np.pi * np.outer(n, n) / N)
    F1r = F1.real.astype(np.float32)
    F1i = F1.imag.astype(np.float32)
    T = np.exp(-2j * np.pi * np.outer(n, n) / L)
    Tr = T.real.astype(np.float32)
    Ti = T.imag.astype(np.float32)
    f16 = lambda x: x.astype(np.float16)
    c = {}
    c["f1ri"] = np.concatenate([F1r, F1i], 1)  # (128,256) f32, FWD1 moving
    c["f2s"] = f16(np.concatenate([F1r, F1i, -F1i], 1))  # FWD2 stationaries [F2r|F2i|nF2i]
    c["f2mov"] = f16(np.concatenate([F1r, -F1i, F1i, F1r], 1))  # (128,512) INV1 moving
    c["tta"] = f16(np.tile(np.concatenate([Tr, Ti], 1), (1, 2)))  # (128,512) [Tr|Ti|Tr|Ti]
    c["ttb"] = f16(np.tile(np.concatenate([Ti, Tr], 1), (1, 2)))
    c["tia"] = f16(np.tile(np.concatenate([Tr, Ti], 1), (1, 2)) / N)
    c["tib"] = f16(np.tile(np.concatenate([Ti, Tr], 1), (1, 2)) / N)
    c["f1s"] = f16(np.concatenate([F1r, F1i], 1))  # INV2 stationaries
    c["ident"] = np.eye(N, dtype=np.float32)
    return c


def _build_nc(repeat=1):
    import concourse.bass as bass  # noqa: F401
    import concourse.mybir as mybir
    import concourse.tile as tile
    from concourse import bacc

    F32, F32R, F16 = mybir.dt.float32, mybir.dt.float32r, mybir.dt.float16
    MUL = mybir.AluOpType.mult
    COPY = mybir.ActivationFunctionType.Copy
    TANH = mybir.ActivationFunctionType.Tanh

    nc = bacc.Bacc("TRN2", target_bir_lowering=False, debug=False, num_devices=8)

    u_d = nc.dram_tensor("u", [B, HSH, L], F32R, kind="ExternalInput")
    k_d = nc.dram_tensor("k", [HSH, L], F32R, kind="ExternalInput")
    db_d = nc.dram_tensor("db", [N, HSH], F32, kind="ExternalInput")
    ident_d = nc.dram_tensor("ident", [N, N], F32R, kind="ExternalInput")
    f1ri_d = nc.dram_tensor("f1ri", [N, 256], F32R, kind="ExternalInput")
    f2s_d = nc.dram_tensor("f2s", [N, 384], F16, kind="ExternalInput")
    f2mov_d = nc.dram_tensor("f2mov", [N, 512], F16, kind="ExternalInput")
    tta_d = nc.dram_tensor("tta", [N, 512], F16, kind="ExternalInput")
    ttb_d = nc.dram_tensor("ttb", [N, 512], F16, kind="ExternalInput")
    tia_d = nc.dram_tensor("tia", [N, 512], F16, kind="ExternalInput")
    tib_d = nc.dram_tensor("tib", [N, 512], F16, kind="ExternalInput")
    f1s_d = nc.dram_tensor("f1s", [N, 256], F16, kind="ExternalInput")
    y_d = nc.dram_tensor("y", [B, HSH, L], F32, kind="ExternalOutput")

    u_hb = u_d.rearrange("b h (p c) -> h p b c", p=N)
    k_all = k_d.rearrange("h (p c) -> p h c", p=N)
    y_hb = y_d.rearrange("b h (p c) -> h p b c", p=N)

    from contextlib import ExitStack

    with tile.TileContext(nc) as tc:
        with ExitStack() as stack:
            ep = stack.enter_context
            cp = ep(tc.tile_pool(name="const", bufs=1))
            ekp = ep(tc.tile_pool(name="ekp", bufs=1))
            ekrepp = ep(tc.tile_pool(name="ekrepp", bufs=2))
            dip = ep(tc.tile_pool(name="dip", bufs=2))
            pa = ep(tc.tile_pool(name="ap", bufs=4))
            pdt16 = ep(tc.tile_pool(name="dt16p", bufs=4))
            pm = ep(tc.tile_pool(name="mp", bufs=4))
            pct = ep(tc.tile_pool(name="ctp", bufs=4))
            pet16 = ep(tc.tile_pool(name="et16p", bufs=4))
            pq = ep(tc.tile_pool(name="qp", bufs=4))
            ppt = ep(tc.tile_pool(name="ptp", bufs=4))
            pg16 = ep(tc.tile_pool(name="g16p", bufs=4))
            pr = ep(tc.tile_pool(name="rp", bufs=4))
            ph = ep(tc.tile_pool(name="hp", bufs=4))
            py = ep(tc.tile_pool(name="yp", bufs=4))
            pdt_ps = ep(tc.tile_pool(name="dtps", bufs=2, space="PSUM"))
            pet_ps = ep(tc.tile_pool(name="etps", bufs=3, space="PSUM"))
            pg_ps = ep(tc.tile_pool(name="gps", bufs=2, space="PSUM"))
            py_ps = ep(tc.tile_pool(name="yps", bufs=1, space="PSUM"))
            # ---- load constants ----
            c_f1ri = cp.tile([N, 256], F32R)
            nc.sync.dma_start(c_f1ri[:], f1ri_d[:])
            c_f2s = cp.tile([N, 384], F16)
            nc.sync.dma_start(c_f2s[:], f2s_d[:])
            c_f2mov = cp.tile([N, 512], F16)
            nc.sync.dma_start(c_f2mov[:], f2mov_d[:])
            c_tta = cp.tile([N, 512], F16)
            nc.sync.dma_start(c_tta[:], tta_d[:])
            c_ttb = cp.tile([N, 512], F16)
            nc.sync.dma_start(c_ttb[:], ttb_d[:])
            c_tia = cp.tile([N, 512], F16)
            nc.sync.dma_start(c_tia[:], tia_d[:])
            c_tib = cp.tile([N, 512], F16)
            nc.sync.dma_start(c_tib[:], tib_d[:])
            c_f1s = cp.tile([N, 256], F16)
            nc.sync.dma_start(c_f1s[:], f1s_d[:])
            c_ident = cp.tile([N, N], F32R)
            nc.sync.dma_start(c_ident[:], ident_d[:])
            c_db = cp.tile([N, HSH], F32)
            nc.sync.dma_start(c_db[:], db_d[:])
            ek = ekp.tile([N, HSH * 256], F16)  # per-h spectra [EkR|EkI], scaled 1/128

            from contextlib import nullcontext
            rep_ctx = tc.For_i(0, repeat, 1) if repeat > 1 else nullcontext()

            def fwd_pair(a, dt_ps, et_ps):
                """FWD FFT for 2 seqs in a (128,256) f32. Leaves [Er0|Ei0|Er1|Ei1] in et_ps."""
                # FWD1: DT = A.T @ [F1r|F1i]  (fp32r full-rate)
                nc.tensor.matmul(dt_ps[:, 0:256], a[:, 0:N],
                                 c_f1ri[:], start=True, stop=True)
                nc.tensor.matmul(dt_ps[:, 256:512], a[:, N:256],
                                 c_f1ri[:], start=True, stop=True)
                dt16 = pdt16.tile([N, 512], F16)
                nc.scalar.activation(dt16[:], dt_ps[:], COPY)
                m1 = pm.tile([N, 512], F16)
                m2 = pm.tile([N, 512], F16)
                nc.vector.tensor_tensor(m1[:], dt16[:], c_tta[:], MUL)
                nc.vector.tensor_tensor(m2[:], dt16[:], c_ttb[:], MUL)
                ct = pct.tile([N, 512], F16)  # [CTr0|CTi0|CTr1|CTi1]
                m1_4 = m1[:].rearrange("p (s t c) -> p s t c", s=2, t=2)
                m2_4 = m2[:].rearrange("p (s t c) -> p s t c", s=2, t=2)
                ct4 = ct[:].rearrange("p (s t c) -> p s t c", s=2, t=2)
                nc.vector.tensor_sub(ct4[:, :, 0, :], m1_4[:, :, 0, :], m1_4[:, :, 1, :])
                nc.vector.tensor_add(ct4[:, :, 1, :], m2_4[:, :, 0, :], m2_4[:, :, 1, :])
                # FWD2: per-seq contiguous groups (start=True clears has_written
                # for the WHOLE bank, so groups sharing a bank must not interleave).
                # Er = F2r@CTr - F2i@CTi ; Ei = F2r@CTi + F2i@CTr
                for s in range(2):
                    o, cb = 256 * s, 256 * s
                    nc.tensor.matmul(et_ps[:, o:o + 256], c_f2s[:, 0:N],
                                     ct[:, cb:cb + 256], start=True, stop=False)
                    nc.tensor.matmul(et_ps[:, o:o + N], c_f2s[:, 256:384],
                                     ct[:, cb + N:cb + 256], start=False, stop=True)
                    nc.tensor.matmul(et_ps[:, o + N:o + 256], c_f2s[:, N:256],
                                     ct[:, cb:cb + N], start=False, stop=True)

            # ---- phase 1: k spectra (one DMA for all 32 k rows) ----
            stack.enter_context(rep_ctx)
            k_sb = ekp.tile([N, HSH * N], F32R)
            nc.sync.dma_start(k_sb[:].rearrange("p (h c) -> p h c", h=HSH), k_all[:])
            for hp in range(HSH // 2):
                a = k_sb[:, hp * 256:(hp + 1) * 256]
                dt_ps = pdt_ps.tile([N, 512], F32)
                et_ps = pet_ps.tile([N, 512], F32)
                fwd_pair(a, dt_ps, et_ps)
                nc.scalar.activation(ek[:, hp * 512:(hp + 1) * 512], et_ps[:], COPY,
                                     scale=1.0 / N)

            # ---- phase 2: u pipeline ----
            for h in range(HSH):
                ekrep = ekrepp.tile([N, 768], F16)  # [R|I|R|I|R|I]
                for t in range(3):
                    nc.vector.tensor_copy(ekrep[:, t * 256:(t + 1) * 256],
                                          ek[:, h * 256:(h + 1) * 256])
                dI = dip.tile([N, N], F32R)
                nc.vector.tensor_scalar_mul(dI[:], c_ident[:], c_db[:, h:h + 1])
                u_h = pa.tile([N, B * N], F32R)
                nc.sync.dma_start(u_h[:].rearrange("p (b c) -> p b c", b=B), u_hb[h])
                y_h = py.tile([N, B * N], F32)
                for bp in range(B // 2):
                    a = u_h[:, bp * 256:(bp + 1) * 256]
                    dt_ps = pdt_ps.tile([N, 512], F32)
                    et_ps = pet_ps.tile([N, 512], F32)
                    fwd_pair(a, dt_ps, et_ps)
                    et16 = pet16.tile([N, 512], F16)
                    nc.scalar.activation(et16[:], et_ps[:], COPY)
                    # spectrum product (GPSIMD)
                    q1 = pq.tile([N, 512], F16)
                    q2 = pq.tile([N, 512], F16)
                    nc.vector.tensor_tensor(q1[:], et16[:], ekrep[:, 0:512], MUL)
                    nc.vector.tensor_tensor(q2[:], et16[:], ekrep[:, 128:640], MUL)
                    pt = ppt.tile([N, 512], F16)  # [PTr0|PTi0|PTr1|PTi1]
                    q1_4 = q1[:].rearrange("p (s t c) -> p s t c", s=2, t=2)
                    q2_4 = q2[:].rearrange("p (s t c) -> p s t c", s=2, t=2)
                    pt4 = pt[:].rearrange("p (s t c) -> p s t c", s=2, t=2)
                    nc.gpsimd.tensor_sub(pt4[:, :, 0, :], q1_4[:, :, 0, :], q1_4[:, :, 1, :])
                    nc.gpsimd.tensor_add(pt4[:, :, 1, :], q2_4[:, :, 0, :], q2_4[:, :, 1, :])
                    # INV1: stationary = data (PTr/PTi), moving = packed F2c consts
                    g_ps = pg_ps.tile([N, 512], F32)
                    nc.tensor.matmul(g_ps[:, 0:256], pt[:, 0:N], c_f2mov[:, 0:256],
                                     start=True, stop=False)
                    nc.tensor.matmul(g_ps[:, 0:256], pt[:, N:256], c_f2mov[:, 256:512],
                                     start=False, stop=True)
                    nc.tensor.matmul(g_ps[:, 256:512], pt[:, 256:384], c_f2mov[:, 0:256],
                                     start=True, stop=False)
                    nc.tensor.matmul(g_ps[:, 256:512], pt[:, 384:512], c_f2mov[:, 256:512],
                                     start=False, stop=True)
                    g16 = pg16.tile([N, 512], F16)
                    nc.scalar.activation(g16[:], g_ps[:], COPY)
                    # inverse twiddle (DVE mults, GPSIMD adds)
                    r1 = pr.tile([N, 512], F16)
                    r2 = pr.tile([N, 512], F16)
                    nc.vector.tensor_tensor(r1[:], g16[:], c_tia[:], MUL)
                    nc.vector.tensor_tensor(r2[:], g16[:], c_tib[:], MUL)
                    hsb = ph.tile([N, 512], F16)  # [Hr0|Hr1|Hi0|Hi1]
                    r1_4 = r1[:].rearrange("p (s t c) -> p s t c", s=2, t=2)
                    r2_4 = r2[:].rearrange("p (s t c) -> p s t c", s=2, t=2)
                    h4 = hsb[:].rearrange("p (t s c) -> p t s c", t=2, s=2)
                    nc.gpsimd.tensor_add(h4[:, 0, :, :], r1_4[:, :, 0, :], r1_4[:, :, 1, :])
                    nc.gpsimd.tensor_sub(h4[:, 1, :, :], r2_4[:, :, 1, :], r2_4[:, :, 0, :])
                    # INV2 + diag(D) feedthrough
                    y_ps = py_ps.tile([N, 256], F32)
                    nc.tensor.matmul(y_ps[:], c_f1s[:, 0:N], hsb[:, 0:256],
                                     start=True, stop=False)
                    nc.tensor.matmul(y_ps[:], c_f1s[:, N:256], hsb[:, 256:512],
                                     start=False, stop=False)
                    nc.tensor.matmul(y_ps[:], dI[:], a[:],
                                     start=False, stop=True)
                    nc.scalar.activation(y_h[:, bp * 256:(bp + 1) * 256], y_ps[:], TANH)
                nc.sync.dma_start(y_hb[h], y_h[:].rearrange("p (b c) -> p b c", b=B))

    nc.finalize()
    return nc


def kernel(u, k, D, **_ignore):
    from concourse.bass_utils import run_bass_kernel_spmd

    u = np.ascontiguousarray(u, dtype=np.float32)
    k = np.ascontiguousarray(k, dtype=np.float32)
    D = np.ascontiguousarray(D, dtype=np.float32)

    if "nc" not in _CACHE:
        _CACHE["nc"] = _build_nc()
    nc = _CACHE["nc"]

    c = _consts()
    d = np.diag(D).astype(np.float32)
    in_maps = []
    for core in range(8):
        h0 = core * HSH
        db = np.tile(d[h0:h0 + HSH][None, :], (N, 1)).astype(np.float32)
        m = {
            "u": u[:, h0:h0 + HSH, :],
            "k": k[h0:h0 + HSH, :],
            "db": db,
        }
        for name in ("f1ri", "f2s", "f2mov", "tta", "ttb", "tia", "tib", "f1s", "ident"):
            m[name] = c[name]
        in_maps.append(m)

    res = run_bass_kernel_spmd(nc, in_maps, core_ids=list(range(8)),
                               **_CACHE.get("run_kwargs", {}))
    _CACHE["last_result"] = res
    y = np.concatenate([res.results[core]["y"] for core in range(8)], axis=1)
    return y



# revision 2
# speedup vs baseline: 50271.9157x; 50271.9157x over previous
"""Trainium2 Bass kernel for FFTConv: y = tanh(ifft(fft(u)*fft(k)).real + diag(D)*u).

Shapes: u (8,256,16384) f32, k (256,16384) f32, D (256,256) f32.

Strategy vs baseline:
- Shard H across 8 cores (32 channels each).
- Complex batch-packing: pair batch rows (b0,b1) into z = u[b0] + i*u[b1]; the
  whole conv pipeline is C-linear, so y[b0] = Re(out), y[b1] = Im(out). Halves
  FFT matmul and pointwise work per sequence.
- diag(D) feedthrough folded into the k-spectrum on host: Ek' = (FFT(k)+d)/128,
  since IFFT(U*d)/L = d*u. Kills the diag matmul.
- k spectra computed on host (fp64 FFT), shipped as fp16 [k2,k1] tiles with the
  layout [EkR | EkI | -EkR] so the product stage is 2 mults + 1 fused subtract.
- All twiddle consts carry sign flips so every complex-mult stage is
  2 DVE mults + 1 strided "R-half minus I-half" subtract (no separate add).
- fp16 I/O with host-side pretranspose to [h, p, b, c]: per-h DMAs are fully
  contiguous (128 descriptors instead of 1024) - the baseline was descriptor
  bound on SP.SEQ.
- 10-slot software pipeline across h so every cross-engine dependency is at
  least one iteration old: PE never stalls; Act/DVE/Pool run ~parallel.
"""

import numpy as np

B, H, L = 8, 256, 16384
N = 128
HSH = H // 8  # 32 channels per core
NPAIR = B // 2  # 4 complex-packed batch pairs
FD = NPAIR * 256  # 1024: free width of the per-h working tiles

_CACHE = {}


def _consts():
    n = np.arange(N)
    F1 = np.exp(-2j * np.pi * np.outer(n, n) / N)
    F1r = F1.real.astype(np.float32)
    F1i = F1.imag.astype(np.float32)
    T = np.exp(-2j * np.pi * np.outer(n, n) / L)
    Tr = T.real.astype(np.float32)
    Ti = T.imag.astype(np.float32)
    f16 = lambda x: np.ascontiguousarray(x.astype(np.float16))
    c = {}
    # FWD1 moving: [DTr|DTi] = Ar@[F1r|F1i] + Ai@[-F1i|F1r]
    c["f1a"] = f16(np.concatenate([F1r, F1i], 1))  # (128,256)
    c["f1b"] = f16(np.concatenate([-F1i, F1r], 1))  # (128,256)
    # FWD2/INV2 stationaries: [F1r | F1i | -F1i]
    c["f2s"] = f16(np.concatenate([F1r, F1i, -F1i], 1))  # (128,384)
    # INV1 moving: [Gr|Gi] = ptR@[F1r|-F1i] + ptI@[F1i|F1r]
    c["ia"] = f16(np.concatenate([F1r, -F1i], 1))  # (128,256)
    c["ib"] = f16(np.concatenate([F1i, F1r], 1))  # (128,256)
    # fwd twiddle (x4 pair-tiled), sign-flipped for fused R-I subtract:
    # ctR = m1R - m1I with m1 = dt*[Tr|Ti];  ctI = m2R - m2I with m2 = dt*[Ti|-Tr]
    c["tta"] = f16(np.tile(np.concatenate([Tr, Ti], 1), (1, NPAIR)))  # (128,1024)
    c["ttb"] = f16(np.tile(np.concatenate([Ti, -Tr], 1), (1, NPAIR)))
    # inv twiddle with 1/N: hR = r1R - r1I, r1 = g*[Tr|-Ti]/N
    #                       hI = r2R - r2I, r2 = g*[-Ti|-Tr]/N
    c["tia"] = f16(np.tile(np.concatenate([Tr, -Ti], 1), (1, NPAIR)) / N)
    c["tib"] = f16(np.tile(np.concatenate([-Ti, -Tr], 1), (1, NPAIR)) / N)
    return c


def _build_nc(repeat=1):
    import concourse.bass as bass  # noqa: F401
    import concourse.mybir as mybir
    import concourse.tile as tile
    from concourse import bacc
    from contextlib import ExitStack, nullcontext

    F32, F16 = mybir.dt.float32, mybir.dt.float16
    MUL = mybir.AluOpType.mult
    COPY = mybir.ActivationFunctionType.Copy
    TANH = mybir.ActivationFunctionType.Tanh

    nc = bacc.Bacc("TRN2", target_bir_lowering=False, debug=False, num_devices=8)

    u_d = nc.dram_tensor("u", [HSH, N, FD], F16, kind="ExternalInput")
    ek_d = nc.dram_tensor("ek", [HSH, N, 384], F16, kind="ExternalInput")
    f1a_d = nc.dram_tensor("f1a", [N, 256], F16, kind="ExternalInput")
    f1b_d = nc.dram_tensor("f1b", [N, 256], F16, kind="ExternalInput")
    f2s_d = nc.dram_tensor("f2s", [N, 384], F16, kind="ExternalInput")
    ia_d = nc.dram_tensor("ia", [N, 256], F16, kind="ExternalInput")
    ib_d = nc.dram_tensor("ib", [N, 256], F16, kind="ExternalInput")
    tta_d = nc.dram_tensor("tta", [N, FD], F16, kind="ExternalInput")
    ttb_d = nc.dram_tensor("ttb", [N, FD], F16, kind="ExternalInput")
    tia_d = nc.dram_tensor("tia", [N, FD], F16, kind="ExternalInput")
    tib_d = nc.dram_tensor("tib", [N, FD], F16, kind="ExternalInput")
    y_d = nc.dram_tensor("y", [HSH, N, FD], F16, kind="ExternalOutput")

    with tile.TileContext(nc) as tc:
        with ExitStack() as stack:
            ep = stack.enter_context
            cp = ep(tc.tile_pool(name="const", bufs=1))
            pu = ep(tc.tile_pool(name="up", bufs=3))
            pdt16 = ep(tc.tile_pool(name="dt16p", bufs=2))
            pm12 = ep(tc.tile_pool(name="m12p", bufs=2))
            pct = ep(tc.tile_pool(name="ctp", bufs=2))
            pet16 = ep(tc.tile_pool(name="et16p", bufs=2))
            pq12 = ep(tc.tile_pool(name="q12p", bufs=2))
            ppt = ep(tc.tile_pool(name="ptp", bufs=2))
            pg16 = ep(tc.tile_pool(name="g16p", bufs=2))
            pr12 = ep(tc.tile_pool(name="r12p", bufs=2))
            phs = ep(tc.tile_pool(name="hsp", bufs=2))
            py16 = ep(tc.tile_pool(name="y16p", bufs=2))
            pdt_ps = ep(tc.tile_pool(name="dtps", bufs=1, space="PSUM"))
            pet_ps = ep(tc.tile_pool(name="etps", bufs=1, space="PSUM"))
            pg_ps = ep(tc.tile_pool(name="gps", bufs=1, space="PSUM"))
            py_ps = ep(tc.tile_pool(name="yps", bufs=1, space="PSUM"))

            # ---- constants ----
            c_f1a = cp.tile([N, 256], F16)
            nc.sync.dma_start(c_f1a[:], f1a_d[:])
            c_f1b = cp.tile([N, 256], F16)
            nc.sync.dma_start(c_f1b[:], f1b_d[:])
            c_f2s = cp.tile([N, 384], F16)
            nc.sync.dma_start(c_f2s[:], f2s_d[:])
            c_ia = cp.tile([N, 256], F16)
            nc.sync.dma_start(c_ia[:], ia_d[:])
            c_ib = cp.tile([N, 256], F16)
            nc.sync.dma_start(c_ib[:], ib_d[:])
            c_tta = cp.tile([N, FD], F16)
            nc.sync.dma_start(c_tta[:], tta_d[:])
            c_ttb = cp.tile([N, FD], F16)
            nc.sync.dma_start(c_ttb[:], ttb_d[:])
            c_tia = cp.tile([N, FD], F16)
            nc.sync.dma_start(c_tia[:], tia_d[:])
            c_tib = cp.tile([N, FD], F16)
            nc.sync.dma_start(c_tib[:], tib_d[:])
            ek_sb = cp.tile([N, HSH * 384], F16)

            rep_ctx = tc.For_i(0, repeat, 1) if repeat > 1 else nullcontext()
            stack.enter_context(rep_ctx)

            ts = {}  # per-h live tiles

            def fused_sub(eng, src, dst):
                """dst[j, {R,I}, c] = src-half0 - src-half1 per pair.
                src is [p, 2048] = [m1(1024) | m2(1024)], each [pairs of R|I].
                dst is [p, 1024] = per pair [R(128) | I(128)]."""
                sv = src[:].rearrange("p (s j t c) -> p s j t c", s=2, j=NPAIR, t=2)
                dv = dst[:].rearrange("p (j s c) -> p s j c", j=NPAIR, s=2)
                eng.tensor_sub(dv, sv[:, :, :, 0, :], sv[:, :, :, 1, :])

            nh = HSH
            for i in range(nh + 10):
                # --- d0: loads ---
                if i < nh:
                    u_h = pu.tile([N, FD], F16)
                    nc.sync.dma_start(u_h[:], u_d[i])
                    nc.sync.dma_start(ek_sb[:, i * 384:(i + 1) * 384], ek_d[i])
                    ts[i] = {"u": u_h}
                # --- d1: FWD1 (PE) + dt16 copy (Act) ---
                h = i - 1
                if 0 <= h < nh:
                    t = ts[h]
                    dt_ps = pdt_ps.tile([N, FD], F32)
                    u_h = t.pop("u")
                    for j in range(NPAIR):
                        o = j * 256
                        nc.tensor.matmul(dt_ps[:, o:o + 256], u_h[:, o:o + N],
                                         c_f1a[:], start=True, stop=False)
                        nc.tensor.matmul(dt_ps[:, o:o + 256], u_h[:, o + N:o + 256],
                                         c_f1b[:], start=False, stop=True)
                    dt16 = pdt16.tile([N, FD], F16)
                    nc.scalar.activation(dt16[:], dt_ps[:], COPY)
                    t["dt16"] = dt16
                # --- d2: fwd twiddle mults (DVE) ---
                h = i - 2
                if 0 <= h < nh:
                    t = ts[h]
                    dt16 = t.pop("dt16")
                    m12 = pm12.tile([N, 2 * FD], F16)
                    nc.vector.tensor_tensor(m12[:, 0:FD], dt16[:], c_tta[:], MUL)
                    nc.vector.tensor_tensor(m12[:, FD:2 * FD], dt16[:], c_ttb[:], MUL)
                    t["m12"] = m12
                # --- d3: ct fused sub (Pool) ---
                h = i - 3
                if 0 <= h < nh:
                    t = ts[h]
                    ct = pct.tile([N, FD], F16)
                    fused_sub(nc.gpsimd, t.pop("m12"), ct)
                    t["ct"] = ct
                # --- d4: FWD2 (PE) + et16 copy (Act) ---
                h = i - 4
                if 0 <= h < nh:
                    t = ts[h]
                    ct = t.pop("ct")
                    et_ps = pet_ps.tile([N, FD], F32)
                    for j in range(NPAIR):
                        o = j * 256
                        nc.tensor.matmul(et_ps[:, o:o + 256], c_f2s[:, 0:N],
                                         ct[:, o:o + 256], start=True, stop=False)
                        nc.tensor.matmul(et_ps[:, o:o + N], c_f2s[:, 256:384],
                                         ct[:, o + N:o + 256], start=False, stop=True)
                        nc.tensor.matmul(et_ps[:, o + N:o + 256], c_f2s[:, N:256],
                                         ct[:, o:o + N], start=False, stop=True)
                    et16 = pet16.tile([N, FD], F16)
                    nc.scalar.activation(et16[:], et_ps[:], COPY)
                    t["et16"] = et16
                # --- d5: spectrum product mults (DVE) ---
                h = i - 5
                if 0 <= h < nh:
                    t = ts[h]
                    et16 = t.pop("et16")
                    q12 = pq12.tile([N, 2 * FD], F16)
                    e0 = h * 384
                    etv = et16[:].rearrange("p (j c) -> p j c", j=NPAIR)
                    ek1 = ek_sb[:, e0:e0 + 256].unsqueeze(1).broadcast_to([N, NPAIR, 256])
                    ek2 = ek_sb[:, e0 + 128:e0 + 384].unsqueeze(1).broadcast_to(
                        [N, NPAIR, 256])
                    q1v = q12[:, 0:FD].rearrange("p (j c) -> p j c", j=NPAIR)
                    q2v = q12[:, FD:2 * FD].rearrange("p (j c) -> p j c", j=NPAIR)
                    nc.vector.tensor_tensor(q1v, etv, ek1, MUL)
                    nc.vector.tensor_tensor(q2v, etv, ek2, MUL)
                    t["q12"] = q12
                # --- d6: pt fused sub (Pool) ---
                h = i - 6
                if 0 <= h < nh:
                    t = ts[h]
                    pt = ppt.tile([N, FD], F16)
                    fused_sub(nc.gpsimd, t.pop("q12"), pt)
                    t["pt"] = pt
                # --- d7: INV1 (PE) + g16 copy (Act) ---
                h = i - 7
                if 0 <= h < nh:
                    t = ts[h]
                    pt = t.pop("pt")
                    g_ps = pg_ps.tile([N, FD], F32)
                    for j in range(NPAIR):
                        o = j * 256
                        nc.tensor.matmul(g_ps[:, o:o + 256], pt[:, o:o + N],
                                         c_ia[:], start=True, stop=False)
                        nc.tensor.matmul(g_ps[:, o:o + 256], pt[:, o + N:o + 256],
                                         c_ib[:], start=False, stop=True)
                    g16 = pg16.tile([N, FD], F16)
                    nc.scalar.activation(g16[:], g_ps[:], COPY)
                    t["g16"] = g16
                # --- d8: inv twiddle mults + hsb fused sub (DVE) ---
                h = i - 8
                if 0 <= h < nh:
                    t = ts[h]
                    g16 = t.pop("g16")
                    r12 = pr12.tile([N, 2 * FD], F16)
                    nc.vector.tensor_tensor(r12[:, 0:FD], g16[:], c_tia[:], MUL)
                    nc.vector.tensor_tensor(r12[:, FD:2 * FD], g16[:], c_tib[:], MUL)
                    hsb = phs.tile([N, FD], F16)
                    fused_sub(nc.vector, r12, hsb)
                    t["hsb"] = hsb
                # --- d9: INV2 (PE) + tanh (Act) ---
                h = i - 9
                if 0 <= h < nh:
                    t = ts[h]
                    hsb = t.pop("hsb")
                    y_ps = py_ps.tile([N, FD], F32)
                    for j in range(NPAIR):
                        o = j * 256
                        nc.tensor.matmul(y_ps[:, o:o + 256], c_f2s[:, 0:N],
                                         hsb[:, o:o + 256], start=True, stop=False)
                        nc.tensor.matmul(y_ps[:, o:o + N], c_f2s[:, N:256],
                                         hsb[:, o + N:o + 256], start=False, stop=True)
                        nc.tensor.matmul(y_ps[:, o + N:o + 256], c_f2s[:, 256:384],
                                         hsb[:, o:o + N], start=False, stop=True)
                    y16 = py16.tile([N, FD], F16)
                    nc.scalar.activation(y16[:], y_ps[:], TANH)
                    t["y16"] = y16
                # --- d10: store ---
                h = i - 10
                if 0 <= h < nh:
                    nc.sync.dma_start(y_d[h], ts[h].pop("y16")[:])
                    del ts[h]

    nc.finalize()
    return nc


def kernel(u, k, D, **_ignore):
    from concourse.bass_utils import run_bass_kernel_spmd

    u = np.asarray(u, dtype=np.float32)
    k = np.asarray(k, dtype=np.float32)
    D = np.asarray(D, dtype=np.float32)

    if "nc" not in _CACHE:
        _CACHE["nc"] = _build_nc()
    nc = _CACHE["nc"]

    c = _consts()
    d = np.diag(D).astype(np.float64)
    K = np.fft.fft(k.astype(np.float64), axis=-1)  # (256,16384) c128
    Kd = (K + d[:, None]) / N

    in_maps = []
    for core in range(8):
        h0 = core * HSH
        uc = u[:, h0:h0 + HSH, :].reshape(B, HSH, N, N).transpose(1, 2, 0, 3)
        uc = np.ascontiguousarray(uc, dtype=np.float16).reshape(HSH, N, FD)
        Kc = Kd[h0:h0 + HSH].reshape(HSH, N, N)  # [h, k2, k1]
        ekc = np.ascontiguousarray(
            np.concatenate([Kc.real, Kc.imag, -Kc.real], axis=2).astype(np.float16))
        m = {"u": uc, "ek": ekc}
        for name in ("f1a", "f1b", "f2s", "ia", "ib", "tta", "ttb", "tia", "tib"):
            m[name] = c[name]
        in_maps.append(m)

    res = run_bass_kernel_spmd(nc, in_maps, core_ids=list(range(8)),
                               **_CACHE.get("run_kwargs", {}))
    _CACHE["last_result"] = res
    ys = []
    for core in range(8):
        yc = res.results[core]["y"].reshape(HSH, N, B, N)
        ys.append(yc.transpose(2, 0, 1, 3).reshape(B, HSH, L))
    return np.concatenate(ys, axis=1).astype(np.float32)
